# revision 42
# baseline (speedup 1.0000x reference)
"""Bass/Trainium2 kernel for a 4-layer GraphSAGE GNN (mean aggregation).

Problem (hardcoded): N=100000 nodes, E=1200000 edges, x:[N,3] f32,
edge_index:[2,E] int64, hidden=64, out=2, log_softmax output.

  h1 = relu(mean_nbr(x) @ Wl1 + x @ Wr1 + b1)
  h2 = relu(mean_nbr(h1) @ Wl2 + h1 @ Wr2 + b2)
  h3 = relu(mean_nbr(h2) @ Wl3 + h2 @ Wr3 + b3)
  out = log_softmax(mean_nbr(h3) @ Wl4 + h3 @ Wr4 + b4)

Strategy (8 NeuronCores, node-partitioned, v2 = gather + PE aggregation):
- Core k owns nodes [k*12544, (k+1)*12544). Per layer each core gathers its
  in-edge source rows (256B) from the allgathered node table with
  dma_gather, 4 streams keyed by src chunk (table quarter, so indices fit
  int16), one SWDGE queue per stream for ring-level parallelism.
- No scatter: aggregation is matmul. Edges are pre-sorted by dst tile
  (128 dst nodes); for each tile a PSUM tile [64 feat, 128 dst]
  accumulates  G_g.T @ Sw_g  over the tile's edge groups, where G_g is
  the gathered bf16 feature block (partitions = edges) and Sw_g is a
  host-built bf16 selection matrix with 1/deg folded in. The result IS
  the feature-major mean, so the epilogue needs no transpose and no
  div-by-degree.
- Epilogue per 512 nodes: 3 PSUM matmuls (Wl/Wr/bias-rank-1), relu on
  ACT; h kept feature-major in a DRAM ping-pong for the Wr matmul and
  node-major in the allgathered table for the next layer's gather.
- Layer 1 aggregates T1 = x @ Wl1 (so gather rows are 256B even though
  x rows are only 12B); layer 1's mean-part matmul is an identity.
- Group structure (edge counts per (tile, chunk) run, padded to x64) is
  shared across cores (max over cores) so all 8 cores run one program;
  shorter cores pad with zero-weight edges.
"""

import os
import numpy as np
from contextlib import ExitStack

# debug/bench switches (unset in normal use)
_SKIP_EDGE = os.environ.get("K_SKIP_EDGE", "") != ""
_SKIP_MM = os.environ.get("K_SKIP_MM", "") != ""
_SKIP_CC = os.environ.get("K_SKIP_CC", "") != ""
_SKIP_EPI = os.environ.get("K_SKIP_EPI", "") != ""
_MM_CONST = os.environ.get("K_MM_CONST", "") != ""   # matmuls vs const rhs, no Sw dma
_SW_ONLY = os.environ.get("K_SW_ONLY", "") != ""     # Sw dmas only, no matmuls

# ---- problem constants (self-contained; do not read spec/reference) ----
N = 100000
E = 1200000
NCORES = 8
NPC = -(-N // (NCORES * 128)) * 128  # nodes per core = 12544 = 98 * 128
NPAD = NCORES * NPC         # 100352
NCHUNK = 4
CH = NPAD // NCHUNK         # 25088 table rows per chunk (int16-safe)
HALF = NPC // 2             # 6272 rows: half-shard unit for split allgathers
F = 64
FIN = 3
FOUT = 2
SUB = int(os.environ.get("K_SUB", "1024"))  # edges per gather call
SCRATCH = 16384 if SUB <= 1024 else 32 * SUB  # SWDGE ring carveout (2 calls/queue)
GROUP = 512                 # nodes per epilogue group
NBLK = NPC // 128           # tiles per core = 98
GRAN = int(os.environ.get("K_GRAN", "64"))  # run padding granularity (64|128)

_CACHE = {}


def _wrap_idx(idx: np.ndarray) -> np.ndarray:
    """Edge i -> idxs[i%16, i//16], replicated for the 8 Q7 cores."""
    w = idx.reshape(-1, 16).T.astype(np.int16)
    return np.tile(w, (8, 1))


def _group_schedule(pad_tc):
    """Shared group schedule from the padded (tile, chunk) run lengths.

    Returns (groups, ncalls) where groups[t] is a list of
    (c, call_idx, slice_idx, p0, gsz, colbase) in accumulation order and
    colbase assigns each group 128 Sw columns, tile-major.
    """
    nt = pad_tc.shape[0]
    off = [0] * NCHUNK
    groups = [[] for _ in range(nt)]
    run_off = np.zeros((nt, NCHUNK), np.int64)
    colbase = 0
    for t in range(nt):
        for c in range(NCHUNK):
            run_off[t, c] = off[c]
            rem = int(pad_tc[t, c])
            while rem > 0:
                o = off[c]
                # runs start x128-aligned, so p0 is always 0 (the PE rejects
                # partition-offset-64 operands on HW)
                assert o % 128 == 0
                gsz = 128 if rem >= 128 else 64
                groups[t].append((c, o // SUB, (o % SUB) // 128, 0, gsz, colbase))
                colbase += 128
                off[c] += gsz
                rem -= gsz
            off[c] = -(-off[c] // 128) * 128
    lens = [int(-(-off[c] // SUB)) for c in range(NCHUNK)]
    return groups, lens, run_off, colbase


def _preprocess(x: np.ndarray, edge_index: np.ndarray):
    import ml_dtypes

    src = np.asarray(edge_index[0], dtype=np.int64)
    dst = np.asarray(edge_index[1], dtype=np.int64)

    deg = np.bincount(dst, minlength=NPAD).astype(np.float32)
    invdeg = (1.0 / np.maximum(deg, 1.0)).astype(np.float32)

    owner = dst // NPC
    # Degree-balanced node placement: within each core, deal nodes to the 98
    # tiles snake-wise by in-degree so per-(tile,chunk) edge counts are
    # nearly equal across tiles AND cores -- this shrinks the shared
    # (max-over-cores) run padding. pi[k] maps original local id -> table
    # position; host unpermutes the output rows at the end.
    pi = np.empty((NCORES, NPC), np.int64)
    for k in range(NCORES):
        degk = deg[k * NPC : (k + 1) * NPC]
        order = np.argsort(-degk, kind="stable")
        i = np.arange(NPC)
        r, j = i // NBLK, i % NBLK
        tile = np.where(r % 2 == 0, j, NBLK - 1 - j)
        posn = np.empty(NPC, np.int64)
        # within-tile rank = count of earlier i in same tile = r
        posn[order] = tile * 128 + r
        pi[k] = posn
    # global table position of every node
    tpos = pi[np.arange(NPAD) // NPC, np.arange(NPAD) % NPC]

    # table position: half h = tpos//HALF selects TA/TB; chunk c = h*2 +
    # (owner>=4), local row = (owner%4)*HALF + tpos%HALF (int16, < 25088)
    per_core = []
    counts = np.zeros((NCORES, NBLK, NCHUNK), np.int64)
    for k in range(NCORES):
        m = owner == k
        s_k = src[m]
        d_k = pi[k][dst[m] - k * NPC]  # dst in table order
        inv_k = invdeg[dst[m]]
        t_k = d_k >> 7
        s_o = s_k // NPC
        s_tp = tpos[s_k]
        c_k = (s_tp // HALF) * 2 + (s_o >= 4)
        s_loc = (s_o % 4) * HALF + (s_tp % HALF)
        key = t_k * NCHUNK + c_k
        order = np.argsort(key, kind="stable")
        s_loc, d_k, c_k, key, inv_k = (
            s_loc[order], d_k[order], c_k[order], key[order], inv_k[order]
        )
        cnt = np.bincount(key, minlength=NBLK * NCHUNK).reshape(NBLK, NCHUNK)
        counts[k] = cnt
        per_core.append((s_loc, d_k, c_k, key, inv_k))

    maxc = counts.max(axis=0)
    pad_tc = -(-maxc // GRAN) * GRAN  # ceil to run granularity
    # every tile needs >= 1 group so its PSUM gets start/stop
    empty = pad_tc.sum(axis=1) == 0
    pad_tc[empty, 0] = GRAN

    groups, ncalls, run_off, swcols = _group_schedule(pad_tc)
    # equalize call counts across chunks: the module emits gather calls in
    # strict 0,1,2,3 queue round-robin so the tile DMASW semaphore lanes
    # (8, assigned round-robin) each see a single SWDGE queue.
    ncalls = [max(ncalls)] * NCHUNK
    L = [ncalls[c] * SUB for c in range(NCHUNK)]

    # group col lookup: for (t, c) run, the Sw column base of each x64 unit
    # unit u of run (t,c) belongs to group index: rebuild per run from groups
    gidx_maps, sw_maps = [], []
    for k in range(NCORES):
        s_loc, d_k, c_k, key, inv_k = per_core[k]
        cnt = counts[k]
        # position of each edge within its (t,c) run
        run_start_of_key = np.zeros(NBLK * NCHUNK, np.int64)
        np.cumsum(cnt.reshape(-1)[:-1], out=run_start_of_key[1:])
        rank = np.arange(len(s_loc)) - run_start_of_key[key]
        # stream position = run_off[t,c] + rank
        pos = run_off[d_k >> 7, c_k] + rank

        # gather index streams
        streams = []
        for c in range(NCHUNK):
            st = np.zeros(L[c], np.int64)
            mc = c_k == c
            st[pos[mc]] = s_loc[mc]
            streams.append(st)
        gidx_maps.append(_wrap_idx(np.concatenate(streams)))

        # Sw: [128, swcols] f32 -> bf16; entry at [pos%128, colbase+slot]
        sw = np.zeros((128, swcols), np.float32)
        # per-edge column base: group of unit (rank//64) of run (t,c)
        # build unit->colbase map per (t,c)
        unit_cb = np.zeros((NBLK, NCHUNK, int(pad_tc.max()) // 64), np.int64)
        for t in range(NBLK):
            off_c = {c: 0 for c in range(NCHUNK)}
            for (c, ci, sl, p0, gsz, cb) in groups[t]:
                u0 = off_c[c] // 64
                for u in range(gsz // 64):
                    unit_cb[t, c, u0 + u] = cb
                off_c[c] += gsz
        tt = d_k >> 7
        cb_e = unit_cb[tt, c_k, rank // 64]
        slot = d_k & 127
        sw[pos % 128, cb_e + slot] = inv_k
        sw_maps.append(sw.astype(ml_dtypes.bfloat16))

    # per-core transposed features [fin, NPC], columns in table order
    xpad = np.zeros((NPAD, FIN), np.float32)
    xpad[:N] = x
    xT = []
    for k in range(NCORES):
        invpi = np.argsort(pi[k])
        xT.append(
            np.ascontiguousarray(xpad[k * NPC : (k + 1) * NPC][invpi].T)
        )
    # out_cat[k*NPC + pi[k][n]] holds node k*NPC+n
    outperm = (np.arange(NPAD) // NPC) * NPC + tpos
    meta = dict(
        pad_tc=pad_tc, groups=groups, ncalls=ncalls, swcols=swcols, L=L,
        outperm=outperm,
    )
    return meta, gidx_maps, sw_maps, xT


def _build_module(meta):
    import concourse.bass as bass
    import concourse.bacc as bacc
    import concourse.mybir as mybir
    from concourse import tile
    from concourse import library_config
    from concourse import masks

    f32 = mybir.dt.float32
    bf16 = mybir.dt.bfloat16
    i16 = mybir.dt.int16
    AF = mybir.ActivationFunctionType
    ALU = mybir.AluOpType

    groups, ncalls, swcols = meta["groups"], meta["ncalls"], meta["swcols"]
    L = meta["L"]
    LG = sum(L) // 16
    stream_base = [sum(L[:c]) for c in range(NCHUNK)]
    nc = bacc.Bacc(
        None,
        target_bir_lowering=False,
        num_swdge_queues=4,
        dynamic_dma_scratch_size=SCRATCH,
    )

    # ---- parameters ----
    xT_p = nc.declare_dram_parameter("xT", [FIN, NPC], f32, isOutput=False)
    gidx_p = nc.declare_dram_parameter("gidx", [128, LG], i16, isOutput=False)
    sw_p = nc.declare_dram_parameter("Sw", [128, swcols], bf16, isOutput=False)
    wl_p, wr_p, b_p = [None], [None], [None]
    for l in range(1, 5):
        din = FIN if l == 1 else F
        dout = FOUT if l == 4 else F
        wl_p.append(nc.declare_dram_parameter(f"Wl{l}", [din, dout], f32, isOutput=False))
        wr_p.append(nc.declare_dram_parameter(f"Wr{l}", [din, dout], f32, isOutput=False))
        b_p.append(nc.declare_dram_parameter(f"b{l}", [1, dout], f32, isOutput=False))
    out_p = nc.declare_dram_parameter("out_shard", [NPC, FOUT], f32, isOutput=True)

    # ---- internal DRAM ----
    # per-layer table split in half-shards: TA = all cores' first half-shards
    # (chunks 0,1), TB = second half-shards (chunks 2,3); each filled by its
    # own allgather so next-layer gathers can start when their half arrives
    TAB = [None] + [
        [
            nc.dram_tensor(f"T{l}{s}", [NCORES * HALF, F], f32, addr_space="Shared")
            for s in "AB"
        ]
        for l in range(1, 5)
    ]
    shAB = [None] + [
        [nc.dram_tensor(f"sh{l}{s}", [HALF, F], f32) for s in "AB"]
        for l in range(1, 5)
    ]
    hTd = [nc.dram_tensor(f"hT{i}", [F, NPC], f32) for i in range(2)]  # ping-pong
    BB = HALF // 128  # tile index of the half boundary

    # epilogue groups: (start_block, n_blocks)
    egroups = []
    b0 = 0
    while b0 < NBLK:
        nb = min(GROUP // 128, NBLK - b0)
        egroups.append((b0, nb))
        b0 += nb

    with tile.TileContext(nc) as tc, ExitStack() as ctx:
        idxp = ctx.enter_context(tc.tile_pool(name="idx", bufs=1))
        constp = ctx.enter_context(tc.tile_pool(name="const", bufs=1))
        gtp = [
            ctx.enter_context(tc.tile_pool(name=f"gt{c}", bufs=2))
            for c in range(NCHUNK)
        ]
        gbp = [
            ctx.enter_context(tc.tile_pool(name=f"gb{c}", bufs=3))
            for c in range(NCHUNK)
        ]
        swp = ctx.enter_context(tc.tile_pool(name="sw", bufs=3))
        grpp = ctx.enter_context(tc.tile_pool(name="grp", bufs=3))
        psA = ctx.enter_context(tc.tile_pool(name="psA", bufs=2, space="PSUM"))
        psB = ctx.enter_context(tc.tile_pool(name="psB", bufs=2, space="PSUM"))
        psC = ctx.enter_context(tc.tile_pool(name="psC", bufs=2, space="PSUM"))

        nc.gpsimd.load_library(library_config.mlp)

        # ---- persistent constants ----
        gi = idxp.tile([128, LG], i16)
        nc.sync.dma_start(gi[:], gidx_p[:])

        ident = constp.tile([128, 128], f32)
        masks.make_identity(nc, ident[:])
        ones = constp.tile([1, GROUP], f32)
        nc.vector.memset(ones[:], 1.0)
        swconst = None
        if _MM_CONST:
            swconst = constp.tile([128, 128], bf16, tag="swconst")
            nc.vector.memset(swconst[:], 0.0)

        wl_t, wr_t, b_t = [None], [None], [None]
        for l in range(1, 5):
            din = FIN if l == 1 else F
            dout = FOUT if l == 4 else F
            t1 = constp.tile([din, dout], f32, tag=f"wl{l}")
            t2 = constp.tile([din, dout], f32, tag=f"wr{l}")
            t3 = constp.tile([1, dout], f32, tag=f"b{l}")
            nc.sync.dma_start(t1[:], wl_p[l][:])
            nc.sync.dma_start(t2[:], wr_p[l][:])
            nc.sync.dma_start(t3[:], b_p[l][:])
            wl_t.append(t1)
            wr_t.append(t2)
            b_t.append(t3)

        def write_sh(lp, g0, nb, hnm):
            """Write node-major h blocks to the split shard halves."""
            a_sp = max(0, min(nb, BB - g0))
            if a_sp > 0:
                nc.sync.dma_start(
                    shAB[lp][0][g0 * 128 : (g0 + a_sp) * 128, :].rearrange(
                        "(a p) f -> p a f", p=128
                    ),
                    hnm[:, :a_sp, :],
                )
            if a_sp < nb:
                nc.sync.dma_start(
                    shAB[lp][1][
                        (g0 + a_sp) * 128 - HALF : (g0 + nb) * 128 - HALF, :
                    ].rearrange("(a p) f -> p a f", p=128),
                    hnm[:, a_sp:nb, :],
                )

        def cc_half(lp, s):
            nc.gpsimd.collective_compute(
                "AllGather",
                mybir.AluOpType.bypass,
                replica_groups=[list(range(NCORES))],
                ins=[shAB[lp][s][:]],
                outs=[TAB[lp][s][:]],
            )

        qc = [0]  # global gather-emission counter: queue = qc % 4 keeps each
        # tile DMASW lane (8, round-robin) on a single SWDGE queue

        _REPS = int(os.environ.get("K_REPS", "1"))
        for _rep in range(_REPS):
            # ---- layer-1 table: T1 = x @ Wl1, node-major, then allgather ----
            ccA = False
            for g0, nb in egroups:
                xt = grpp.tile([FIN, GROUP], f32, tag="prevT")
                nc.sync.dma_start(
                    xt[:, : nb * 128], xT_p[:, g0 * 128 : (g0 + nb) * 128]
                )
                pnm = psC.tile([128, GROUP // 128, F], f32, tag="nm")
                for a in range(nb):
                    nc.tensor.matmul(
                        pnm[:, a, :],
                        xt[:, a * 128 : (a + 1) * 128],
                        wl_t[1][:],
                        start=True,
                        stop=True,
                    )
                hnm = grpp.tile([128, GROUP // 128, F], f32, tag="hnm")
                nc.scalar.activation(hnm[:, :nb, :], pnm[:, :nb, :], AF.Copy)
                write_sh(1, g0, nb, hnm)
                if not _SKIP_CC and not ccA and g0 + nb >= BB:
                    cc_half(1, 0)
                    ccA = True
            # half-B allgather is emitted inside layer 1's gather loop (after
            # the chunk 0/1 calls) so those don't queue behind it on Pool
            pending_B = 1 if not _SKIP_CC else None

            # ---- layers ----
            for l in range(1, 5):
                din = FIN if l == 1 else F
                dout = FOUT if l == 4 else F
                prev_dram = xT_p if l == 1 else hTd[l % 2]
                next_hT = hTd[(l + 1) % 2]

                # gather + bf16 convert; chunk-major emission so chunks 0,1
                # (table half A) flow while half B's allgather completes
                gb_tiles = [[None] * ncalls[c] for c in range(NCHUNK)]
                for c in range(NCHUNK):
                    if c == 2 and pending_B is not None:
                        cc_half(pending_B, 1)
                        pending_B = None
                    for i in range(ncalls[c]):
                        if _SKIP_EDGE:
                            continue
                        tbl = TAB[l][c // 2][(c % 2) * CH : (c % 2 + 1) * CH, :]
                        gbase = stream_base[c] + i * SUB
                        gt = gtp[c].tile([128, SUB // 128, F], f32, tag=f"e{c}")
                        nc.gpsimd.dma_gather(
                            gt[:],
                            tbl,
                            gi[:, gbase // 16 : (gbase + SUB) // 16],
                            SUB,
                            SUB,
                            F,
                            queue_num=qc[0] % 4,
                        )
                        qc[0] += 1
                        gb = gbp[c].tile([128, SUB // 128, F], bf16, tag=f"b{c}")
                        nc.scalar.activation(gb[:], gt[:], AF.Copy)
                        gb_tiles[c][i] = gb

                # aggregation matmuls + epilogue per 512-node group
                ccA = False
                for g0, nb in egroups:
                    if _SKIP_EPI and l < 4:
                        continue
                    rows = slice(g0 * 128, (g0 + nb) * 128)
                    mt = grpp.tile([F, GROUP], f32, tag="mt")
                    for a in range(nb):
                        t = g0 + a
                        pmt = psA.tile([F, 128], f32, tag="agg")
                        if _SKIP_EDGE or _SKIP_MM:
                            nc.vector.memset(mt[:, a * 128 : (a + 1) * 128], 0.0)
                            continue
                        # per-tile Sw columns
                        cb0 = groups[t][0][5]
                        cb1 = groups[t][-1][5] + 128
                        if not _MM_CONST:
                            swt = swp.tile([128, cb1 - cb0], bf16, tag="sw")
                            nc.sync.dma_start(swt[:], sw_p[:, cb0:cb1])
                        if _SW_ONLY:
                            nc.vector.memset(mt[:, a * 128 : (a + 1) * 128], 0.0)
                            continue
                        ng = len(groups[t])
                        for j, (c, ci, sl, p0, gsz, cb) in enumerate(groups[t]):
                            gb = gb_tiles[c][ci]
                            rhs = (
                                swconst[p0 : p0 + gsz, :]
                                if _MM_CONST
                                else swt[p0 : p0 + gsz, cb - cb0 : cb - cb0 + 128]
                            )
                            nc.tensor.matmul(
                                pmt[:],
                                gb[p0 : p0 + gsz, sl, :],
                                rhs,
                                start=(j == 0),
                                stop=(j == ng - 1),
                            )
                        nc.scalar.activation(
                            mt[:, a * 128 : (a + 1) * 128], pmt[:], AF.Copy
                        )

                    # prev features (feature-major) for the Wr part
                    pv = grpp.tile([din, GROUP], f32, tag="prevT")
                    nc.sync.dma_start(pv[:, : nb * 128], prev_dram[:, rows])

                    ph = psB.tile([dout, GROUP], f32, tag="h")
                    if l == 1:
                        nc.tensor.matmul(
                            ph[:, : nb * 128],
                            ident[:F, :F],
                            mt[:, : nb * 128],
                            start=True,
                            stop=False,
                        )
                    else:
                        nc.tensor.matmul(
                            ph[:, : nb * 128],
                            wl_t[l][:],
                            mt[:, : nb * 128],
                            start=True,
                            stop=False,
                        )
                    nc.tensor.matmul(
                        ph[:, : nb * 128],
                        wr_t[l][:],
                        pv[:, : nb * 128],
                        start=False,
                        stop=False,
                    )
                    nc.tensor.matmul(
                        ph[:, : nb * 128],
                        b_t[l][:],
                        ones[:, : nb * 128],
                        start=False,
                        stop=True,
                    )

                    if l < 4:
                        hT_sb = grpp.tile([F, GROUP], f32, tag="hT_sb")
                        nc.scalar.activation(
                            hT_sb[:, : nb * 128], ph[:, : nb * 128], AF.Relu
                        )
                        nc.sync.dma_start(next_hT[:, rows], hT_sb[:, : nb * 128])
                        # node-major for the next table
                        pnm = psC.tile([128, GROUP // 128, F], f32, tag="nm")
                        for a in range(nb):
                            nc.tensor.transpose(
                                pnm[:, a, :],
                                hT_sb[:, a * 128 : (a + 1) * 128],
                                ident[:F, :F],
                            )
                        hnm = grpp.tile([128, GROUP // 128, F], f32, tag="hnm")
                        nc.vector.tensor_copy(hnm[:, :nb, :], pnm[:, :nb, :])
                        write_sh(l + 1, g0, nb, hnm)
                        if (
                            not (_SKIP_CC or _SKIP_EPI)
                            and not ccA
                            and g0 + nb >= BB
                        ):
                            cc_half(l + 1, 0)
                            ccA = True
                    else:
                        # logits -> node-major -> log_softmax -> out_shard
                        zsb = grpp.tile([FOUT, GROUP], f32, tag="zsb")
                        nc.vector.tensor_copy(zsb[:, : nb * 128], ph[:, : nb * 128])
                        pz = psC.tile([128, GROUP // 128, FOUT], f32, tag="znm")
                        for a in range(nb):
                            nc.tensor.transpose(
                                pz[:, a, :],
                                zsb[:, a * 128 : (a + 1) * 128],
                                ident[:FOUT, :FOUT],
                            )
                        z = grpp.tile([128, GROUP // 128, FOUT], f32, tag="z")
                        nc.vector.tensor_copy(z[:, :nb, :], pz[:, :nb, :])
                        z0 = z[:, :nb, 0:1]
                        z1 = z[:, :nb, 1:2]
                        m = grpp.tile([128, GROUP // 128, 1], f32, tag="m")
                        nc.vector.tensor_tensor(m[:, :nb, :], z0, z1, ALU.max)
                        d = grpp.tile([128, GROUP // 128, FOUT], f32, tag="d")
                        nc.vector.tensor_tensor(d[:, :nb, 0:1], z0, m[:, :nb, :], ALU.subtract)
                        nc.vector.tensor_tensor(d[:, :nb, 1:2], z1, m[:, :nb, :], ALU.subtract)
                        e = grpp.tile([128, GROUP // 128, FOUT], f32, tag="e")
                        nc.scalar.activation(e[:, :nb, :], d[:, :nb, :], AF.Exp)
                        s = grpp.tile([128, GROUP // 128, 1], f32, tag="s")
                        nc.vector.tensor_tensor(
                            s[:, :nb, :], e[:, :nb, 0:1], e[:, :nb, 1:2], ALU.add
                        )
                        ls = grpp.tile([128, GROUP // 128, 1], f32, tag="ls")
                        nc.scalar.activation(ls[:, :nb, :], s[:, :nb, :], AF.Ln)
                        o = grpp.tile([128, GROUP // 128, FOUT], f32, tag="o")
                        nc.vector.tensor_tensor(
                            o[:, :nb, 0:1], d[:, :nb, 0:1], ls[:, :nb, :], ALU.subtract
                        )
                        nc.vector.tensor_tensor(
                            o[:, :nb, 1:2], d[:, :nb, 1:2], ls[:, :nb, :], ALU.subtract
                        )
                        nc.sync.dma_start(
                            out_p[rows, :].rearrange("(a p) f -> p a f", p=128),
                            o[:, :nb, :],
                        )

                if l < 4 and not (_SKIP_CC or _SKIP_EPI):
                    pending_B = l + 1

    nc.compile()
    return nc


def _prepare(inputs):
    x = np.asarray(inputs["x"], dtype=np.float32)
    edge_index = np.asarray(inputs["edge_index"])
    meta, gidx_maps, sw_maps, xT = _preprocess(x, edge_index)

    key = (
        meta["pad_tc"].tobytes(),
        _SKIP_EDGE,
        _SKIP_MM,
        _SKIP_CC,
        _SKIP_EPI,
        _MM_CONST,
        _SW_ONLY,
        GRAN,
        SUB,
        os.environ.get("K_REPS", "1"),
    )
    if key not in _CACHE:
        _CACHE[key] = _build_module(meta)
    nc = _CACHE[key]

    in_maps = []
    for k in range(NCORES):
        m = {
            "xT": xT[k],
            "gidx": gidx_maps[k],
            "Sw": sw_maps[k],
        }
        for l in range(1, 5):
            m[f"Wl{l}"] = np.asarray(inputs[f"Wl{l}"], np.float32)
            m[f"Wr{l}"] = np.asarray(inputs[f"Wr{l}"], np.float32)
            m[f"b{l}"] = np.asarray(inputs[f"b{l}"], np.float32).reshape(1, -1)
        in_maps.append(m)
    return nc, in_maps, meta["outperm"]


def _run(inputs, trace=False):
    from concourse.bass_utils import run_bass_kernel_spmd

    nc, in_maps, outperm = _prepare(inputs)
    r = run_bass_kernel_spmd(nc, in_maps, list(range(NCORES)), trace=trace)
    out = np.concatenate(
        [r.results[k]["out_shard"] for k in range(NCORES)], axis=0
    )[outperm][:N]
    return out.astype(np.float32), r


def kernel(**inputs) -> np.ndarray:
    out, _ = _run(inputs)
    return out
